# revision 11
# baseline (speedup 1.0000x reference)
"""Deformable self-attention (retrieval KNN) Trainium2 Bass kernel.

Problem (hardcoded): B=1, N=2048, C=256, H=8 heads, K=4 sample points/head,
C_=32 channels/head, KNN over the 2048 real positions with 4 neighbors and
Shepard interpolation.

Sharding: one head per NeuronCore (8 cores).  Each core computes its head's
sampling locations, brute-force KNN via Max8/MaxIndex8 on the vector engine,
Shepard-weighted value interpolation via the GPSIMD ap_gather against an
SBUF-resident v.T table, then the per-head outputs are all-gathered and each
core computes the output projection for its 1/8 slice of the tokens.

The KNN scores are computed with the exact same instruction recipe the
XLA-compiled reference uses on this hardware (K=2 fp32 PE matmul with samp
stationary, then scale/bias combine + add), so the selected neighbor
indices match the reference's top_k bit-for-bit (up to ties).
"""

import functools

import numpy as np

import concourse.bass as bass
import concourse.mybir as mybir
import concourse.tile as tile
from concourse import bacc
from concourse.bass_utils import run_bass_kernel_spmd

F32 = mybir.dt.float32
U32 = mybir.dt.uint32
I16 = mybir.dt.int16
AX = mybir.AxisListType
ALU = mybir.AluOpType
ACTF = mybir.ActivationFunctionType

B, N, C = 1, 2048, 256
H, K = 8, 4
C_ = C // H  # 32
NT = N // 128  # 16 token tiles
G = 4 * K  # 16 gathers per token
NTC = NT // 8  # token tiles per core in the output projection


def _build_nc(dbg=False, timing=False):
    ndev = 1 if timing else 8
    nc = bacc.Bacc("TRN2", debug=False, target_bir_lowering=False, num_devices=ndev)

    # ---- I/O -------------------------------------------------------------
    x_t = nc.dram_tensor("x_t", [C, N], F32, kind="ExternalInput")  # x transposed
    ext3_d = nc.dram_tensor("ext3", [3, N], F32, kind="ExternalInput")  # [pos.T;1]
    ppneg_d = nc.dram_tensor("ppneg", [1, N], F32, kind="ExternalInput")  # -(px^2+py^2)
    w_off8 = nc.dram_tensor("w_off8", [C, 2 * K], F32, kind="ExternalInput")
    samp_lhs3 = nc.dram_tensor("samp_lhs3", [3, 2 * K], F32, kind="ExternalInput")
    w_attn_t = nc.dram_tensor("w_attn_t", [C, K], F32, kind="ExternalInput")
    b_attn = nc.dram_tensor("b_attn", [1, K], F32, kind="ExternalInput")
    w_v_t = nc.dram_tensor("w_v_t", [C, C_], F32, kind="ExternalInput")
    b_v_c = nc.dram_tensor("b_v_c", [1, C_], F32, kind="ExternalInput")
    w_out_t = nc.dram_tensor("w_out_t", [C, C], F32, kind="ExternalInput")
    b_out = nc.dram_tensor("b_out", [1, C], F32, kind="ExternalInput")
    sp_rep = nc.dram_tensor("sp_rep", [128, 1], F32, kind="ExternalInput")
    out_d = nc.dram_tensor("out", [N, C], F32, kind="ExternalOutput")

    # internal DRAM
    in_cc = nc.dram_tensor("in_cc", [C_, N], F32)
    ag_out = nc.dram_tensor("ag_out", [C, N], F32, addr_space="Shared")
    samp_scr = nc.dram_tensor("samp_scr", [2 * K, N], F32)  # x rows then y rows
    qqn_scr = nc.dram_tensor("qqn_scr", [K, N], F32)  # -qq rows

    dbg_t = {}
    if dbg:
        for nm, shp, dt_ in [
            ("d_sc0", [128, NT, N], F32),
            ("d_mx", [128, NT, K, 8], F32),
            ("d_mi", [128, NT, K, 8], U32),
        ]:
            dbg_t[nm] = nc.dram_tensor(nm, shp, dt_, kind="ExternalOutput")

    with tile.TileContext(nc) as tc:
        with (
            tc.tile_pool(name="const", bufs=1) as pc,
            tc.tile_pool(name="ringA", bufs=2) as prA,
            tc.tile_pool(name="dscr", bufs=3, space="DRAM") as pdr,
        ):
            # ---- persistent SBUF ----------------------------------------
            xt0 = pc.tile([128, N], F32)  # x.T rows 0:128
            xt1 = pc.tile([128, N], F32)  # x.T rows 128:256
            ext3 = pc.tile([3, N], F32)  # [pos.T ; ones]
            POS2 = pc.tile([2, N], F32)  # pos.T rows
            PPN = pc.tile([128, N], F32)  # -pp replicated
            ones128 = pc.tile([1, 128], F32)
            onesrow = pc.tile([1, N], F32)
            woff0 = pc.tile([128, 2 * K], F32)
            woff1 = pc.tile([128, 2 * K], F32)
            wext = pc.tile([3, 2 * K], F32)
            wat0 = pc.tile([128, K], F32)
            wat1 = pc.tile([128, K], F32)
            batn = pc.tile([1, K], F32)
            wv0 = pc.tile([128, C_], F32)
            wv1 = pc.tile([128, C_], F32)
            bvc = pc.tile([1, C_], F32)
            wo0 = pc.tile([128, C], F32)
            wo1 = pc.tile([128, C], F32)
            bo = pc.tile([1, C], F32)
            QQT = pc.tile([128, NT, K], F32)  # -qq token-major
            EATT = pc.tile([128, NT, K], F32)  # exp(attn logits)
            RATT = pc.tile([128, NT, 1], F32)  # 1/sum(exp)
            SPN = pc.tile([128, 1], F32)  # -(relu(p)+1e-6)
            VTAB = pc.tile([C_, N, 1], F32)  # v.T gather table
            ohT = pc.tile([C_, N], F32)  # per-head out (transposed)
            lf0 = pc.tile([128, N], F32)  # allgathered out.T rows 0:128
            lf1 = pc.tile([128, N], F32)

            nc.sync.dma_start(xt0[:], x_t[0:128, :])
            nc.sync.dma_start(xt1[:], x_t[128:256, :])
            nc.sync.dma_start(ext3[:], ext3_d[:, :])
            nc.sync.dma_start(POS2[:], ext3_d[0:2, :])
            nc.sync.dma_start(
                PPN[:],
                ppneg_d[:, :].rearrange("a b -> (a b)")[None, :].to_broadcast([128, N]),
            )
            nc.vector.memset(ones128[:], 1.0)
            nc.vector.memset(onesrow[:], 1.0)
            nc.sync.dma_start(woff0[:], w_off8[0:128, :])
            nc.sync.dma_start(woff1[:], w_off8[128:256, :])
            nc.sync.dma_start(wext[:], samp_lhs3[:, :])
            nc.sync.dma_start(wat0[:], w_attn_t[0:128, :])
            nc.sync.dma_start(wat1[:], w_attn_t[128:256, :])
            nc.sync.dma_start(batn[:], b_attn[:, :])
            nc.sync.dma_start(wv0[:], w_v_t[0:128, :])
            nc.sync.dma_start(wv1[:], w_v_t[128:256, :])
            nc.sync.dma_start(bvc[:], b_v_c[:, :])
            nc.sync.dma_start(wo0[:], w_out_t[0:128, :])
            nc.sync.dma_start(wo1[:], w_out_t[128:256, :])
            nc.sync.dma_start(bo[:], b_out[:, :])

            # shepard power -> -(relu(p) + 1e-6), replicated [128,1]
            spt = pc.tile([128, 1], F32)
            nc.sync.dma_start(spt[:], sp_rep[:, :])
            spr = pc.tile([128, 1], F32)
            nc.scalar.activation(spr[:], spt[:], ACTF.Relu)
            nc.vector.tensor_scalar(
                SPN[:], spr[:], -1.0, -1e-6, op0=ALU.mult, op1=ALU.add
            )

            # ---- phase A1: samp coords + v.T table ----------------------
            with tc.tile_pool(name="psA1", bufs=1, space="PSUM") as psA1:
                # sampT8 [2K, N]: rows 0:4 = x coords (k major), 4:8 = y
                sampT_ps = psA1.tile([2 * K, N], F32)
                for j in range(4):
                    sl = slice(j * 512, (j + 1) * 512)
                    nc.tensor.matmul(
                        sampT_ps[:, sl], woff0[:], xt0[:, sl], start=True, stop=False
                    )
                    nc.tensor.matmul(
                        sampT_ps[:, sl], woff1[:], xt1[:, sl], start=False, stop=False
                    )
                    nc.tensor.matmul(
                        sampT_ps[:, sl], wext[:], ext3[:, sl], start=False, stop=True
                    )
                SAMPT8 = prA.tile([2 * K, N], F32, bufs=1)
                nc.scalar.copy(SAMPT8[:], sampT_ps[:])
                # bounce through DRAM; pair rows are sliced back per (nt,k)
                nc.sync.dma_start(samp_scr[:, :], SAMPT8[:])
                SX4 = prA.tile([K, N], F32, bufs=1, name="SX4")
                nc.vector.tensor_copy(SX4[:], SAMPT8[0:K, :])
                SY4 = prA.tile([K, N], F32, bufs=1, name="SY4")
                nc.sync.dma_start(SY4[:], samp_scr[K : 2 * K, :])

                # qq = sx*sx + sy*sy per (k, token); then negate
                SQX = prA.tile([K, N], F32, bufs=1)
                nc.vector.tensor_tensor(SQX[:], SX4[:], SX4[:], op=ALU.mult)
                SQY = prA.tile([K, N], F32, bufs=1)
                nc.vector.tensor_tensor(SQY[:], SY4[:], SY4[:], op=ALU.mult)
                QQ4 = prA.tile([K, N], F32, bufs=1)
                nc.vector.tensor_tensor(QQ4[:], SQX[:], SQY[:], op=ALU.add)
                QQ4n = prA.tile([K, N], F32, bufs=1)
                nc.vector.tensor_scalar(QQ4n[:], QQ4[:], -1.0, None, op0=ALU.mult)
                nc.sync.dma_start(qqn_scr[:, :], QQ4n[:])
                # token-major: QQT[p, nt, k] = -qq[k, nt*128+p]
                for k in range(K):
                    nc.sync.dma_start(
                        QQT[:, :, k : k + 1],
                        qqn_scr[k, :].rearrange("(nt p) -> p nt", p=128)[:, :, None],
                    )

                # v.T table: VTAB[c, n] = (x @ W_v_h.T + b_v)[n, c]
                vt_ps = psA1.tile([C_, N], F32)
                for j in range(4):
                    sl = slice(j * 512, (j + 1) * 512)
                    nc.tensor.matmul(
                        vt_ps[:, sl], wv0[:], xt0[:, sl], start=True, stop=False
                    )
                    nc.tensor.matmul(
                        vt_ps[:, sl], wv1[:], xt1[:, sl], start=False, stop=False
                    )
                    nc.tensor.matmul(
                        vt_ps[:, sl], bvc[:], onesrow[:, sl], start=False, stop=True
                    )
                nc.scalar.copy(VTAB[:, :, 0], vt_ps[:])

            # ---- phase A2: attn ----------------------------------------
            with tc.tile_pool(name="psA3", bufs=2, space="PSUM") as psA3:
                # attn: exp + 1/sum per token tile
                for nt in range(NT):
                    sl = slice(nt * 128, (nt + 1) * 128)
                    aps = psA3.tile([128, K], F32, tag="aps")
                    nc.tensor.matmul(aps[:], xt0[:, sl], wat0[:], start=True, stop=False)
                    nc.tensor.matmul(aps[:], xt1[:, sl], wat1[:], start=False, stop=False)
                    nc.tensor.matmul(aps[:], ones128[:], batn[:], start=False, stop=True)
                    nc.scalar.activation(EATT[:, nt, :], aps[:], ACTF.Exp)
                    sat = prA.tile([128, 1], F32, tag="sat")
                    nc.vector.reduce_sum(sat[:], EATT[:, nt, :], axis=AX.X)
                    nc.vector.reciprocal(RATT[:, nt, :], sat[:])

            # ---- phase B/C: scores + selection + gather + interp --------
            with (
                tc.tile_pool(name="psB", bufs=2, space="PSUM") as psB,
                tc.tile_pool(name="sring", bufs=2) as srng,
                tc.tile_pool(name="cring", bufs=2) as crng,
            ):
                for nt in range(NT):
                    mxc = crng.tile([128, K, 8], F32, tag="mxc")
                    mic = crng.tile([128, K, 8], U32, tag="mic")
                    for k in range(K):
                        # ein = samp . pos via K=2 fp32 PE matmul (bit-exact
                        # match of the reference einsum lowering)
                        eps = psB.tile([128, N], F32, tag="eps")
                        lh = srng.tile([2, 128], F32, tag="lh")
                        nc.sync.dma_start(
                            lh[:],
                            samp_scr[k : k + K + 1 : K, nt * 128 : (nt + 1) * 128],
                        )
                        for j in range(4):
                            sl = slice(j * 512, (j + 1) * 512)
                            nc.tensor.matmul(
                                eps[:, sl], lh[:], POS2[:, sl], start=True, stop=True
                            )
                        # score = (2*ein - qq) - pp  ==  -d2, bit-exact
                        sc = srng.tile([128, N], F32, tag="sc")
                        nc.scalar.activation(
                            sc[:],
                            eps[:],
                            ACTF.Identity,
                            bias=QQT[:, nt, k : k + 1],
                            scale=2.0,
                        )
                        nc.vector.tensor_tensor(sc[:], sc[:], PPN[:], op=ALU.add)
                        if dbg and k == 0:
                            nc.sync.dma_start(dbg_t["d_sc0"][:, nt, :], sc[:])
                        nc.vector.max(out=mxc[:, k, :], in_=sc[:])
                        nc.vector.max_index(
                            out=mic[:, k, :], in_max=mxc[:, k, :], in_values=sc[:]
                        )
                    if dbg:
                        nc.sync.dma_start(dbg_t["d_mx"][:, nt, :, :], mxc[:])
                        nc.sync.dma_start(dbg_t["d_mi"][:, nt, :, :], mic[:])

                    # ---- weights cw[t, (k,j)] (token-major) -------------
                    d2c = crng.tile([128, K, 4], F32, tag="d2c")
                    nc.vector.tensor_scalar(
                        d2c[:], mxc[:, :, 0:4], -1.0, 0.0, op0=ALU.mult, op1=ALU.max
                    )
                    dist = crng.tile([128, K, 4], F32, tag="dist")
                    nc.scalar.activation(dist[:], d2c[:], ACTF.Sqrt)
                    enn = crng.tile([128, K, 4], F32, tag="enn")
                    nc.scalar.activation(enn[:], dist[:], ACTF.Exp, scale=SPN[:])
                    se = crng.tile([128, K], F32, tag="se")
                    nc.vector.reduce_sum(se[:], enn[:], axis=AX.X)
                    rnn = crng.tile([128, K], F32, tag="rnn")
                    nc.vector.reciprocal(rnn[:], se[:])
                    a4 = crng.tile([128, K], F32, tag="a4")
                    nc.vector.tensor_scalar(
                        a4[:], EATT[:, nt, :], RATT[:, nt, :], None, op0=ALU.mult
                    )
                    b4 = crng.tile([128, K], F32, tag="b4")
                    nc.vector.tensor_tensor(b4[:], a4[:], rnn[:], op=ALU.mult)
                    cwt = crng.tile([128, K, 4], F32, tag="cwt")
                    nc.vector.tensor_tensor(
                        cwt[:],
                        enn[:],
                        b4[:][:, :, None].to_broadcast([128, K, 4]),
                        op=ALU.mult,
                    )

                    # ---- index wrap + cw replication via DRAM -----------
                    mi16 = crng.tile([128, G], I16, tag="mi16")
                    nc.vector.tensor_copy(
                        mi16[:].rearrange("p (a b) -> p a b", b=4), mic[:, :, 0:4]
                    )
                    scr_i = pdr.tile([128, G], I16, tag="scr_i")
                    nc.sync.dma_start(scr_i[:, :], mi16[:])
                    osb = crng.tile([C_, 128], I16, tag="osb")
                    flat_i = scr_i[:, :].rearrange("a b -> (a b)")
                    for blk in range(C_ // 16):
                        nc.sync.dma_start(
                            osb[16 * blk : 16 * blk + 16, :],
                            flat_i.rearrange("(b a) -> a b", a=16),
                        )
                    scr_w = pdr.tile([128, G], F32, tag="scr_w")
                    nc.sync.dma_start(scr_w[:, :], cwt[:])
                    cw32 = crng.tile([C_, N], F32, tag="cw32")
                    nc.sync.dma_start(
                        cw32[:, :],
                        scr_w[:, :]
                        .rearrange("a b -> (a b)")[None, :]
                        .to_broadcast([C_, N]),
                    )

                    # ---- gather + weighted reduce (channel-major) -------
                    vgt = crng.tile([C_, N, 1], F32, tag="vgt")
                    nc.gpsimd.ap_gather(
                        out_ap=vgt[:],
                        in_ap=VTAB[:],
                        idxs_ap=osb[:],
                        channels=C_,
                        num_elems=N,
                        d=1,
                        num_idxs=N,
                    )
                    nc.vector.tensor_tensor(
                        vgt[:, :, 0], vgt[:, :, 0], cw32[:], op=ALU.mult
                    )
                    nc.vector.reduce_sum(
                        ohT[:, nt * 128 : (nt + 1) * 128],
                        vgt[:, :, 0].rearrange("c (t g) -> c t g", g=G),
                        axis=AX.X,
                    )

            # ---- phase D: allgather + output projection ------------------
            with (
                tc.tile_pool(name="psD", bufs=2, space="PSUM") as psD,
                tc.tile_pool(name="dring", bufs=3) as drng,
            ):
                nc.sync.dma_start(in_cc[:, :], ohT[:])
                if timing:
                    for hh in range(8):
                        nc.sync.dma_start(
                            ag_out[hh * C_ : (hh + 1) * C_, :], in_cc[:, :]
                        )
                else:
                    nc.gpsimd.collective_compute(
                        "AllGather",
                        ALU.bypass,
                        replica_groups=[list(range(8))],
                        ins=[in_cc[:]],
                        outs=[ag_out[:]],
                    )
                nc.sync.dma_start(lf0[:], ag_out[0:128, :])
                nc.sync.dma_start(lf1[:], ag_out[128:256, :])
                for nt in range(NT):
                    sl = slice(nt * 128, (nt + 1) * 128)
                    ops = psD.tile([128, C], F32, tag="ops")
                    nc.tensor.matmul(ops[:], lf0[:, sl], wo0[:], start=True, stop=False)
                    nc.tensor.matmul(ops[:], lf1[:, sl], wo1[:], start=False, stop=False)
                    nc.tensor.matmul(ops[:], ones128[:], bo[:], start=False, stop=True)
                    ot = drng.tile([128, C], F32, tag="ot")
                    nc.scalar.copy(ot[:], ops[:])
                    nc.sync.dma_start(out_d[sl, :], ot[:])

    nc.compile()
    return nc


@functools.lru_cache(maxsize=2)
def _get_nc(dbg=False):
    return _build_nc(dbg=dbg)


def build_in_maps(
    x, pos, W_off, b_off, W_attn, b_attn, W_v, b_v, W_out, b_out, shepard_power
):
    x = np.ascontiguousarray(np.asarray(x, dtype=np.float32))
    pos = np.ascontiguousarray(np.asarray(pos, dtype=np.float32))
    W_off = np.asarray(W_off, dtype=np.float32)
    b_off = np.asarray(b_off, dtype=np.float32)
    W_attn = np.asarray(W_attn, dtype=np.float32)
    b_attn_np = np.asarray(b_attn, dtype=np.float32)
    W_v = np.asarray(W_v, dtype=np.float32)
    b_v_np = np.asarray(b_v, dtype=np.float32)
    W_out = np.asarray(W_out, dtype=np.float32)
    b_out_np = np.asarray(b_out, dtype=np.float32)
    sp = np.float32(np.asarray(shepard_power).reshape(()))

    x2 = x[0]  # [N, C]
    pos2 = pos[0]  # [N, 2]
    x_t = np.ascontiguousarray(x2.T)  # [C, N]
    ext3 = np.ones((3, N), dtype=np.float32)
    ext3[0:2] = pos2.T  # [px ; py ; 1]
    # -pp with the same elementwise recipe the reference uses
    pp = pos2[:, 0] * pos2[:, 0] + pos2[:, 1] * pos2[:, 1]
    ppneg = np.ascontiguousarray((-pp)[None, :])

    # sampT8 row order: x coords k=0..3 then y coords k=0..3
    # p2 pattern adds pos coords; row 2 = b_off
    p2 = np.zeros((2, 2 * K), dtype=np.float32)
    p2[0, 0:K] = 1.0
    p2[1, K : 2 * K] = 1.0

    w_off_h = W_off.reshape(H, K, 2, C)
    b_off_h = b_off.reshape(H, K, 2)
    w_attn_h = W_attn.reshape(H, K, C)
    b_attn_h = b_attn_np.reshape(H, K)
    w_v_h = W_v.reshape(H, C_, C)
    b_v_h = b_v_np.reshape(H, C_)
    w_out_t = np.ascontiguousarray(W_out.T)  # [C, C]

    in_maps = []
    for h in range(H):
        # [C, 8]: cols 0..3 = x rows of k, cols 4..7 = y rows
        w8 = np.concatenate(
            [w_off_h[h, :, 0, :], w_off_h[h, :, 1, :]], axis=0
        ).T  # [C, 8]
        b8 = np.concatenate([b_off_h[h, :, 0], b_off_h[h, :, 1]])  # [8]
        samp_lhs3 = np.concatenate([p2, b8[None, :]], axis=0)  # [3, 8]
        in_maps.append(
            {
                "x_t": x_t,
                "ext3": ext3,
                "ppneg": ppneg,
                "w_off8": np.ascontiguousarray(w8),
                "samp_lhs3": np.ascontiguousarray(samp_lhs3),
                "w_attn_t": np.ascontiguousarray(w_attn_h[h].T),
                "b_attn": np.ascontiguousarray(b_attn_h[h][None, :]),
                "w_v_t": np.ascontiguousarray(w_v_h[h].T),
                "b_v_c": np.ascontiguousarray(b_v_h[h][None, :]),
                "w_out_t": w_out_t,
                "b_out": np.ascontiguousarray(b_out_np[None, :]),
                "sp_rep": np.full((128, 1), sp, dtype=np.float32),
            }
        )
    return in_maps


def kernel(**inputs):
    in_maps = build_in_maps(**inputs)
    nc = _get_nc()
    res = run_bass_kernel_spmd(nc, in_maps, list(range(8)))
    out = res.results[0]["out"]  # [N, C]
    return out.reshape(B, N, C).astype(np.float32)
